# revision 17
# baseline (speedup 1.0000x reference)
"""Trainium2 Bass kernel for DeltaGradientDescent.

reference math:
    x_n   = x / (||x||_2 + eps)                  per row, x: [64, 4096]
    outer = x_n^T x_n / B                        rank-64, [4096, 4096]
    out   = W @ (I - alpha*outer) - lr*G
          = W - (alpha/B) * (W x_n^T) x_n - lr*G

Sharding: W and G row-sharded across 8 cores (512 rows each); x replicated.

Per core the kernel computes the TRANSPOSED output block out_c^T
[4096, 512] so that W only has to be streamed from HBM once, in bf16,
in the transposed packed layout that both consumers need:

    YT    = x @ W_c^T            (PE, rank-64 left factor, PSUM f32)
    ytn   = (-c1 * rinv) . YT    (ACT, fused normalize+scale, bf16)
    xn    = rinv . x             (DVE, per-row normalize, bf16)
    per 128-row chunk jc of out^T (PSUM accumulation):
        ps  = (-lr I) @ G^T_jc   (scaled identity inject: - lr G^T, G in fp8)
        ps += xn_jc^T @ ytn      (rank-64 term: - c1 * Z^T)
        even chunks: ps += I @ W^T_jc (PE inject), ACT copies ps -> bf16
        odd  chunks: DVE moves ps -> bf16 with W^T_jc folded in
                     (scalar_tensor_tensor costs the same as a plain copy)
    out^T staged [128, 4096] bf16, DMA'd out per 8 chunks

All pools live outside the rep loop so buffers rotate ACROSS reps
(cross-rep double buffering); per-rep pools would reuse the same SBUF
addresses and serialize next-rep loads behind this rep's last readers.

The tolerance (2e-2) leaves ~10x margin for bf16 I/O: W/out in bf16
contribute ~1.1e-3 Frobenius rel err each; G rides the lr=1e-3-scaled
term so fp8(e4m3) rounding contributes ~4e-5.

HBM traffic per core: 4 MiB W^T bf16 + 2 MiB G^T fp8 + 1 MiB x/x^T
+ 4 MiB out^T bf16 = 11 MiB (vs 24 MiB for the natural-layout f32
variant), against a ~358 GB/s per-core HBM roofline.
"""

import numpy as np
import ml_dtypes

import concourse.bass as bass
import concourse.mybir as mybir
import concourse.tile as tile
from concourse import bacc
from concourse.bass_utils import run_bass_kernel_spmd

F32 = mybir.dt.float32
BF16 = mybir.dt.bfloat16
FP8 = mybir.dt.float8e4
NP_BF16 = ml_dtypes.bfloat16
NP_FP8 = ml_dtypes.float8_e4m3

DIM = 4096
B = 64
NCORES = 8
R = DIM // NCORES  # 512 rows per core
P = 128
NJC = DIM // P     # 32 column(-of-W) chunks of 128 -> out^T row chunks
NKW = 4            # W^T DMA chunks (1 MiB each)
JCPW = NJC // NKW  # 8 j-chunks per W^T DMA chunk
NKG = 2            # G^T DMA chunks (1 MiB each, fp8)
JCPG = NJC // NKG  # 16 j-chunks per G^T DMA chunk
GRP = 4            # out^T chunks per PSUM group
NGRP = NJC // GRP  # 8 groups
LR = 0.001
ALPHA = 0.01
EPS = 1e-8
C1 = ALPHA / B     # 1.5625e-4

_NC_CACHE = {}


def _build_kernel(tc, pools, wtb, gtb, x, xtb, out, ident1, identg, pmf, pmtf):
    nc = tc.nc
    smalls, wtpool, gtpool, opool, ypsum, npsum, zpool = pools

    # ---- x natural (bf16; norm chain + Z-term stationary) and packed x^T
    #      (bf16; Y^T stationary) — they gate the normalization chain and
    #      the Y^T matmuls
    x_sb = smalls.tile([P, DIM // 2], BF16, tag="x_sb")
    nc.sync.dma_start(out=x_sb, in_=x[:, :])
    xtb_sb = smalls.tile([P, NJC * P], FP8, tag="xtb_sb")
    nc.sync.dma_start(out=xtb_sb, in_=xtb[:, :])

    # ---- bf16 W^T packed chunks (feed Y^T matmuls and the W inject):
    #      wtb_sb[kk][p, c*512 + i] = W_c[i, (kk*8+c)*128 + p]
    wtb_sb = []
    for kk in range(NKW):
        t = wtpool.tile([P, JCPW * R], BF16, tag="wtb_sb")
        nc.sync.dma_start(out=t, in_=wtb[:, kk * JCPW * R : (kk + 1) * JCPW * R])
        wtb_sb.append(t)

    # ---- fp8 G^T packed chunks (only feeds the lr-scaled inject) ----
    gtb_sb = []
    for kk in range(NKG):
        t = gtpool.tile([P, JCPG * R], FP8, tag="gtb_sb")
        nc.sync.dma_start(out=t, in_=gtb[:, kk * JCPG * R : (kk + 1) * JCPG * R])
        gtb_sb.append(t)

    # ---- row norms of x: x rows are split across partition pairs
    #      (b, b+64); per-partition sums are pair-combined with a tiny
    #      pairing-matrix matmul (PE is the only cross-partition path) ----
    nstats = (DIM // 2) // 512
    stats = smalls.tile([P, nstats, 6], F32, tag="stats")
    for si in range(nstats):
        nc.vector.bn_stats(out=stats[:, si, :], in_=x_sb[:, si * 512 : (si + 1) * 512])
    mv = smalls.tile([P, 2], F32, tag="mv")
    nc.vector.bn_aggr(out=mv, in_=stats)
    # per-partition sum(x^2)/2048 = var + mean^2
    msq = smalls.tile([P, 1], F32, tag="msq")
    nc.scalar.activation(out=msq, in_=mv[:, 0:1], func=mybir.ActivationFunctionType.Square)
    ssum = smalls.tile([P, 1], F32, tag="ssum")
    nc.vector.tensor_add(ssum, msq, mv[:, 1:2])
    s2_ps = npsum.tile([P, 1], F32, tag="npsum")
    nc.tensor.matmul(s2_ps[:B, :], lhsT=pmf, rhs=ssum, start=True, stop=True)
    norm = smalls.tile([B, 1], F32, tag="norm")
    nc.scalar.activation(
        out=norm, in_=s2_ps[:B, :], func=mybir.ActivationFunctionType.Sqrt,
        scale=float(DIM // 2),
    )
    nc.vector.tensor_scalar_add(norm, norm, EPS)
    rinv = smalls.tile([B, 1], F32, tag="rinv")
    nc.vector.reciprocal(rinv, norm)
    # replicate rinv to partition pairs (b, b+64) via the transposed pairing
    rp_ps = npsum.tile([P, 1], F32, tag="npsum")
    nc.tensor.matmul(rp_ps, lhsT=pmtf, rhs=rinv, start=True, stop=True)
    rinv_pk = smalls.tile([P, 1], F32, tag="rinv_pk")
    nc.vector.tensor_copy(rinv_pk, rp_ps)
    # -c1 * rinv for the Y^T factor (one factor carries the -c1 scale)
    rinv2 = smalls.tile([P, 1], F32, tag="rinv2")
    nc.vector.tensor_scalar_mul(rinv2, rinv_pk, -C1)
    # normalize x in place (stationary of the rank-64 matmuls)
    xn_sb = x_sb
    nc.vector.tensor_scalar_mul(xn_sb, x_sb, rinv_pk)

    # ---- YT = x @ W_c^T : [64, 512] accumulated over 32 j-chunks ----
    yt_ps = ypsum.tile([P, R], F32, tag="yt_ps")
    for jc in range(NJC):
        nc.tensor.matmul(
            yt_ps,
            lhsT=xtb_sb[:, jc * P : (jc + 1) * P],
            rhs=wtb_sb[jc // JCPW][:, (jc % JCPW) * R : (jc % JCPW + 1) * R],
            start=(jc == 0),
            stop=(jc == NJC - 1),
        )
    # fused normalize + (-c1) scale of the left factor on the PSUM->SBUF
    # copy; runs on ACT (per-partition scale operand) to keep DVE free
    ytn_sb = smalls.tile([P, R], BF16, tag="ytn_sb")
    nc.scalar.activation(
        out=ytn_sb, in_=yt_ps, func=mybir.ActivationFunctionType.Copy, scale=rinv2
    )

    # ---- combine, streamed over groups of 4 out^T chunks [128, 512] ----
    for g2 in range(NGRP // 2):
        o_t = opool.tile([P, 2 * GRP * R], BF16, tag="o_t")
        for gh in range(2):
            g = g2 * 2 + gh
            zs = []
            for q in range(GRP):
                z_ps = zpool.tile([P, R], F32, tag="z_ps")
                zs.append(z_ps)
            # + W^T via PE identity inject on chunk 0 of each group only;
            # the rest get W^T folded into the DVE PSUM->SBUF move (a fused
            # scalar_tensor_tensor costs the same as a plain copy there,
            # and PE is the near-co-bottleneck)
            jc0 = g * GRP
            nc.tensor.matmul(
                zs[0],
                lhsT=ident1,
                rhs=wtb_sb[jc0 // JCPW][:, (jc0 % JCPW) * R : (jc0 % JCPW + 1) * R],
                start=True,
                stop=False,
            )
            # - lr*G^T (one identg LDWEIGHTS for all 4 tiles; G is fp8)
            for q in range(GRP):
                jc = g * GRP + q
                nc.tensor.matmul(
                    zs[q],
                    lhsT=identg,
                    rhs=gtb_sb[jc // JCPG][:, (jc % JCPG) * R : (jc % JCPG + 1) * R],
                    start=(q != 0),
                    stop=False,
                )
            # - c1 * Z^T = xn_jc^T @ ytn  (x cols >= 2048 live on the
            # upper partition half; ytn carries both halves)
            for q in range(GRP):
                jc = g * GRP + q
                hb = (jc // (NJC // 2)) * B
                jl = jc % (NJC // 2)
                nc.tensor.matmul(
                    zs[q],
                    lhsT=xn_sb[hb : hb + B, jl * P : (jl + 1) * P],
                    rhs=ytn_sb[hb : hb + B, :],
                    start=False,
                    stop=True,
                )
            # PSUM -> staging (bf16): chunk 0 plain ACT copy, rest DVE
            # fused (+ W^T) move
            for q in range(GRP):
                jc = g * GRP + q
                dst = o_t[:, (gh * GRP + q) * R : (gh * GRP + q + 1) * R]
                if q == 0:
                    nc.scalar.activation(
                        out=dst, in_=zs[q], func=mybir.ActivationFunctionType.Copy
                    )
                else:
                    nc.vector.scalar_tensor_tensor(
                        out=dst,
                        in0=zs[q],
                        scalar=1.0,
                        in1=wtb_sb[jc // JCPW][:, (jc % JCPW) * R : (jc % JCPW + 1) * R],
                        op0=mybir.AluOpType.mult,
                        op1=mybir.AluOpType.add,
                    )
        nc.scalar.dma_start(
            out=out[:, g2 * 2 * GRP * R : (g2 + 1) * 2 * GRP * R], in_=o_t
        )


def _get_nc(reps=1):
    key = ("nc", reps)
    if key in _NC_CACHE:
        return _NC_CACHE[key]
    from contextlib import ExitStack

    nc = bacc.Bacc(None, target_bir_lowering=False, debug=False)
    wtb = nc.declare_dram_parameter("wtb", [P, NJC * R], BF16, isOutput=False).ap()
    gtb = nc.declare_dram_parameter("gtb", [P, NJC * R], FP8, isOutput=False).ap()
    x = nc.declare_dram_parameter("x", [P, DIM // 2], BF16, isOutput=False).ap()
    xtb = nc.declare_dram_parameter("xtb", [P, NJC * P], FP8, isOutput=False).ap()
    out = nc.declare_dram_parameter("out", [P, NJC * R], BF16, isOutput=True).ap()
    with tile.TileContext(nc) as tc:
        with ExitStack() as octx:
            constp = octx.enter_context(tc.tile_pool(name="constp", bufs=1))
            # pools shared across reps: buffers rotate per tag so rep N+1's
            # loads land in buffers freed one/two reps earlier
            pools = (
                octx.enter_context(tc.tile_pool(name="smalls", bufs=3)),
                octx.enter_context(tc.tile_pool(name="wtpool", bufs=2 * NKW)),
                octx.enter_context(tc.tile_pool(name="gtpool", bufs=2 * NKG)),
                octx.enter_context(tc.tile_pool(name="opool", bufs=4)),
                octx.enter_context(tc.tile_pool(name="ypsum", bufs=1, space="PSUM")),
                octx.enter_context(tc.tile_pool(name="npsum", bufs=1, space="PSUM")),
                octx.enter_context(tc.tile_pool(name="zpool", bufs=6, space="PSUM")),
            )
            # bf16 identity (W inject) and -lr-scaled identity (G inject)
            ident1 = constp.tile([P, P], BF16)
            nc.gpsimd.memset(ident1, 0.0)
            nc.gpsimd.affine_select(
                out=ident1,
                in_=ident1,
                compare_op=mybir.AluOpType.not_equal,
                fill=1.0,
                base=0,
                pattern=[[-1, P]],
                channel_multiplier=1,
            )
            identg = constp.tile([P, P], BF16)
            nc.gpsimd.memset(identg, 0.0)
            nc.gpsimd.affine_select(
                out=identg,
                in_=identg,
                compare_op=mybir.AluOpType.not_equal,
                fill=float(-LR),
                base=0,
                pattern=[[-1, P]],
                channel_multiplier=1,
            )
            # f32 pairing matrices: pmf[p, b] = (p == b) or (p == b + 64);
            # pmtf = pmf^T. Used to pair-combine/replicate across the two
            # partition halves holding each x row.
            pmf = constp.tile([P, B], F32)
            nc.gpsimd.memset(pmf, 0.0)
            for base in (0, -B):
                nc.gpsimd.affine_select(
                    out=pmf, in_=pmf, compare_op=mybir.AluOpType.not_equal,
                    fill=1.0, base=base, pattern=[[-1, B]], channel_multiplier=1,
                )
            pmtf = constp.tile([B, P], F32)
            nc.gpsimd.memset(pmtf, 0.0)
            for base in (0, B):
                nc.gpsimd.affine_select(
                    out=pmtf, in_=pmtf, compare_op=mybir.AluOpType.not_equal,
                    fill=1.0, base=base, pattern=[[-1, P]], channel_multiplier=1,
                )
            for _ in range(reps):
                _build_kernel(tc, pools, wtb, gtb, x, xtb, out, ident1, identg, pmf, pmtf)
    nc.finalize()
    _NC_CACHE[key] = nc
    return nc


def _pack_t(m):
    """[R, DIM] f32 -> packed transpose [P, NJC*R]: out[p, jc*R+i] = m[i, jc*128+p]."""
    return np.ascontiguousarray(
        m.T.reshape(NJC, P, R).transpose(1, 0, 2).reshape(P, NJC * R)
    )


def _unpack_t(o):
    """packed [P, NJC*R] -> [R, DIM]: out[i, jc*128+p] = o[p, jc*R+i]."""
    return o.reshape(P, NJC, R).transpose(2, 1, 0).reshape(R, DIM)


def _make_in_maps(weight, input_x, grad):
    weight = np.asarray(weight, dtype=np.float32)
    grad = np.asarray(grad, dtype=np.float32)
    x = np.ascontiguousarray(np.asarray(input_x, dtype=np.float32))
    # host-side layout packing of x^T into 128-partition chunk form with the
    # 64 batch columns DUPLICATED onto both column halves (fp8): the Y
    # matmuls then produce yt rows on both partition halves directly.
    # xtb[p, jc*128 + t*64 + b] = x[b, jc*128 + p], t in {0, 1}
    xtb0 = x.T.reshape(NJC, P, B).transpose(1, 0, 2)
    xtb = np.ascontiguousarray(
        np.tile(xtb0[:, :, None, :], (1, 1, 2, 1)).reshape(P, NJC * P)
    ).astype(NP_FP8)
    # x rows split across partition pairs (b, b+64): halves stacked
    xb = np.ascontiguousarray(
        np.concatenate([x[:, : DIM // 2], x[:, DIM // 2 :]], axis=0)
    ).astype(NP_BF16)
    in_maps = []
    for c in range(NCORES):
        in_maps.append(
            {
                "wtb": _pack_t(weight[c * R : (c + 1) * R]).astype(NP_BF16),
                "gtb": _pack_t(grad[c * R : (c + 1) * R]).astype(NP_FP8),
                "x": xb,
                "xtb": xtb,
            }
        )
    return in_maps


def run(weight, input_x, grad, trace=False, **kwargs):
    """Run the SPMD kernel; returns (full output, BassKernelResults)."""
    nc = _get_nc()
    in_maps = _make_in_maps(weight, input_x, grad)
    res = run_bass_kernel_spmd(nc, in_maps, list(range(NCORES)), trace=trace, **kwargs)
    out = np.concatenate(
        [
            _unpack_t(np.asarray(res.results[c]["out"])).astype(np.float32)
            for c in range(NCORES)
        ],
        axis=0,
    )
    return out, res


def kernel(weight, input_x, grad):
    out, _ = run(weight, input_x, grad, trace=False)
    return out
